# revision 1
# baseline (speedup 1.0000x reference)
"""Trainium2 Bass kernel for the distributed DCRNN (gnn_message_passing) problem.

Self-contained: host-side preprocessing (node sharding, degree-sorted
destination grids, gather index plumbing) + an SPMD Bass/Tile kernel that
runs on 8 NeuronCores via run_bass_kernel_spmd.
"""

from contextlib import ExitStack

import numpy as np

import concourse.bass as bass
import concourse.bacc as bacc
import concourse.mybir as mybir
import concourse.tile as tile
from concourse.masks import make_identity

P = 128
CH = 16
FILT = 64
SUBW = 1    # grid columns (128 descriptors each) per indirect gather DMA
SUBT = 1    # permute columns per indirect DMA
NQ = 1      # SWDGE queues to spread indirect DMAs across


# ---------------------------------------------------------------------------
# host-side preprocessing (index plumbing only; no reference arithmetic)
# ---------------------------------------------------------------------------

def chunk_plan(D, max_width=192):
    """Split tiles into gather chunks (contiguous tiles, bounded total width)
    and equal-width runs within each chunk.
    Returns list of chunks: (t_lo, t_hi, off_lo, off_hi, runs) with
    runs = [(t_lo, t_hi, D)]."""
    T = len(D)
    off = np.concatenate([[0], np.cumsum(D)]).astype(np.int64)
    chunks = []
    t = 0
    while t < T:
        t0 = t
        w = 0
        while t < T and (w + D[t] <= max_width or t == t0):
            w += D[t]
            t += 1
        runs = []
        r = t0
        while r < t:
            r0 = r
            while r < t and D[r] == D[r0]:
                r += 1
            runs.append((r0, r, int(D[r0])))
        chunks.append((t0, t, int(off[t0]), int(off[t]), runs))
    return chunks


def preprocess(x, edge_index, edge_weight, n_cores=8, sort_degrees=True):
    N = x.shape[0]
    E = edge_index.shape[1]
    NPC = N // n_cores
    T = (NPC + P - 1) // P
    NL = P * T
    row = np.ascontiguousarray(edge_index[0]).astype(np.int64)
    col = np.ascontiguousarray(edge_index[1]).astype(np.int64)
    w = np.ascontiguousarray(edge_weight).astype(np.float32)

    cnt_in = np.bincount(col, minlength=N)
    cnt_out = np.bincount(row, minlength=N)
    if sort_degrees:
        # deal nodes to cores by global in-degree rank (rank % n_cores): all
        # cores then share identical per-tile degree profiles, so the
        # max-over-cores tile-width padding of grid A vanishes
        g_order = np.argsort(cnt_in, kind="stable")
        cores = np.empty(N, dtype=np.int64)
        cores[g_order] = np.arange(N) % n_cores
    else:
        cores = np.arange(N) // NPC

    def make_perm(cnt):
        perm = np.full((n_cores, NL), -1, dtype=np.int64)
        pos = np.empty(N, dtype=np.int64)
        for k in range(n_cores):
            nodes = np.where(cores == k)[0]
            order = np.argsort(cnt[nodes], kind="stable") if sort_degrees \
                else np.arange(NPC)
            perm[k, :NPC] = nodes[order]
            pos[nodes[order]] = np.arange(NPC)
        return perm, pos

    permA, posA = make_perm(cnt_in)
    permB, posB = make_perm(cnt_out)

    def tile_widths(perm, cnt):
        D = np.zeros(T, dtype=np.int64)
        for k in range(n_cores):
            c = np.where(perm[k] >= 0, cnt[np.maximum(perm[k], 0)], 0)
            D = np.maximum(D, c.reshape(T, P).max(axis=1))
        return np.maximum(D, 1)

    DA = tile_widths(permA, cnt_in)
    DB = tile_widths(permB, cnt_out)
    offA = np.concatenate([[0], np.cumsum(DA)]).astype(np.int64)
    offB = np.concatenate([[0], np.cumsum(DB)]).astype(np.int64)
    WA, WB = int(offA[-1]), int(offB[-1])

    jA = posA
    tA, pA = jA // P, jA % P
    table_row = ((cores * P + pA) * T + tA).astype(np.int64)

    jpad = NPC
    if NPC < NL:
        pad_row = (np.arange(n_cores) * P + (jpad % P)) * T + (jpad // P)
    else:
        pad_row = np.zeros(n_cores, dtype=np.int64)

    def build_grid(dest, src, pos, off, W):
        idx = np.zeros((n_cores, P, W), dtype=np.int32)
        wg = np.zeros((n_cores, P, W), dtype=np.float32)
        for k in range(n_cores):
            idx[k, :, :] = pad_row[k]
        k_e = cores[dest]
        j_e = pos[dest]
        t_e, p_e = j_e // P, j_e % P
        order = np.argsort(dest, kind="stable")
        ds = dest[order]
        start = np.concatenate([[0], np.cumsum(np.bincount(ds, minlength=N))])[ds]
        s_e = np.empty(E, dtype=np.int64)
        s_e[order] = np.arange(E) - start
        wcol = off[t_e] + s_e
        idx[k_e, p_e, wcol] = table_row[src]
        wg[k_e, p_e, wcol] = w
        return idx, wg

    idxA, wgA = build_grid(col, row, posA, offA, WA)
    idxB, _wgB_sigma = build_grid(row, col, posB, offB, WB)
    # degree grid for deg_out in pi order directly (weights are host data, so
    # the looser pi-tile padding costs no gather descriptors)
    DBd = tile_widths(permA, cnt_out)
    offBd = np.concatenate([[0], np.cumsum(DBd)]).astype(np.int64)
    WBd = int(offBd[-1])
    _idxBd, wgBd = build_grid(row, col, posA, offBd, WBd)

    jB = posB
    sig_row = ((jB % P) * T + jB // P).astype(np.int64)
    perm_idx = np.zeros((n_cores, P, T), dtype=np.int32)
    scat_idx = np.zeros((n_cores, P, T), dtype=np.int32)
    pad_pi_row = (jpad % P) * T + jpad // P
    for k in range(n_cores):
        pk = permA[k]
        rows = np.where(pk >= 0, sig_row[np.maximum(pk, 0)], 0)
        perm_idx[k] = rows.reshape(T, P).T
        # sigma position (p', t') -> pi DRAM row (p_pi*T + t_pi) of its node
        pkB = permB[k]
        pi_row = np.full(NL, pad_pi_row, dtype=np.int64)
        validB = pkB >= 0
        jA_of = posA[np.maximum(pkB, 0)]
        pi_row[validB] = ((jA_of % P) * T + jA_of // P)[validB]
        scat_idx[k] = pi_row.reshape(T, P).T

    x_grid = np.zeros((n_cores, P, T, CH), dtype=np.float32)
    xT = np.zeros((n_cores, CH, NL), dtype=np.float32)
    for k in range(n_cores):
        pk = permA[k]
        valid = pk >= 0
        xg = np.zeros((NL, CH), dtype=np.float32)
        xg[valid] = x[pk[valid]]
        x_grid[k] = xg.reshape(T, P, CH).transpose(1, 0, 2)
        xT[k] = xg.T

    assert NPC < NL, "need at least one pad slot per core for zero gather rows"
    cfg = dict(
        N=N, E=E, NPC=NPC, T=T, NL=NL, WA=WA, WB=WB, WBd=WBd, n_cores=n_cores,
        chunksA=chunk_plan(DA), chunksB=chunk_plan(DB), chunksBd=chunk_plan(DBd, 10**9),
    )
    arrays = dict(
        idxA=idxA, wgA=wgA, idxB=idxB, wgB=wgBd, perm_idx=perm_idx,
        scat_idx=scat_idx, x_grid=x_grid, xT=xT, permA=permA,
    )
    return cfg, arrays


def make_in_maps(cfg, arrays, w_z, b_z, w_h, b_h, lin_w, lin_b):
    """A^T row layout: [x^T (0:16) | zeros (16:32) | TxO^T (32:48) | TxI^T (48:64)].
    Wcat rows match; rows 16:32 are zero (contraction-dim padding is free)."""
    n_cores = cfg["n_cores"]
    w_id0 = np.concatenate([w_z[0, 0, :CH], w_h[0, 0, :CH]], axis=1).astype(np.float32)
    w_id1 = np.concatenate([w_z[1, 0, :CH], w_h[1, 0, :CH]], axis=1).astype(np.float32)
    w_dif = np.concatenate(
        [np.concatenate([w_z[0, 1, :CH], w_h[0, 1, :CH]], axis=1),
         np.concatenate([w_z[1, 1, :CH], w_h[1, 1, :CH]], axis=1)],
        axis=0).astype(np.float32)
    bias = np.concatenate([b_z, b_h]).astype(np.float32).reshape(P, 1)
    in_maps = []
    for k in range(n_cores):
        in_maps.append({
            "x_grid": np.ascontiguousarray(arrays["x_grid"][k]),
            "xT": np.ascontiguousarray(arrays["xT"][k]),
            "idxA": np.ascontiguousarray(arrays["idxA"][k]),
            "idxB": np.ascontiguousarray(arrays["idxB"][k]),
            "wgA": np.ascontiguousarray(arrays["wgA"][k]),
            "wgB": np.ascontiguousarray(arrays["wgB"][k]),
            "perm_idx": np.ascontiguousarray(arrays["perm_idx"][k]),
            "w_id0": w_id0, "w_id1": w_id1, "w_dif": w_dif,
            "bias": bias,
            "lin_w": lin_w.astype(np.float32),
            "lin_b": lin_b.astype(np.float32).reshape(1, 1),
        })
    return in_maps


def postprocess(cfg, arrays, results):
    """results[k]['out'] is [1, NL]; scatter back to [N, 1] full output."""
    N, NL = cfg["N"], cfg["NL"]
    out = np.zeros((N, 1), dtype=np.float32)
    for k in range(cfg["n_cores"]):
        o = np.asarray(results[k]["out"]).reshape(NL)
        pk = arrays["permA"][k]
        valid = pk >= 0
        out[pk[valid], 0] = o[valid]
    return out


# ---------------------------------------------------------------------------
# device kernel
# ---------------------------------------------------------------------------

def build_kernel(cfg, debug=False):
    T, NL, WA, WB = cfg["T"], cfg["NL"], cfg["WA"], cfg["WB"]
    n_cores = cfg["n_cores"]
    NT = n_cores * NL
    f32 = mybir.dt.float32
    i32 = mybir.dt.int32

    nc = bacc.Bacc(num_swdge_queues=NQ)
    qrr = [0]

    def qnext():
        q = qrr[0]
        qrr[0] = (q + 1) % NQ
        return f"qPoolDynamic{q or ''}"

    dbg = {}
    if debug:
        for name, shape in (("d_degI", [P, T]), ("d_degO", [P, T]),
                            ("d_xs", [P, T, 2 * CH]), ("d_table", [n_cores * NL, 2 * CH]),
                            ("d_TxC", [P, T, 2 * CH]), ("d_AT", [FILT, NL]),
                            ("d_Wcat", [FILT, P])):
            dbg[name] = nc.declare_dram_parameter(name, shape, f32, isOutput=True)

    x_grid_p = nc.declare_dram_parameter("x_grid", [P, T, CH], f32, isOutput=False)
    xT_p = nc.declare_dram_parameter("xT", [CH, NL], f32, isOutput=False)
    idxA_p = nc.declare_dram_parameter("idxA", [P, WA], i32, isOutput=False)
    idxB_p = nc.declare_dram_parameter("idxB", [P, WB], i32, isOutput=False)
    wgA_p = nc.declare_dram_parameter("wgA", [P, WA], f32, isOutput=False)
    wgB_p = nc.declare_dram_parameter("wgB", [P, cfg["WBd"]], f32, isOutput=False)
    perm_p = nc.declare_dram_parameter("perm_idx", [P, T], i32, isOutput=False)
    w_id0_p = nc.declare_dram_parameter("w_id0", [CH, P], f32, isOutput=False)
    w_id1_p = nc.declare_dram_parameter("w_id1", [CH, P], f32, isOutput=False)
    w_dif_p = nc.declare_dram_parameter("w_dif", [2 * CH, P], f32, isOutput=False)
    bias_p = nc.declare_dram_parameter("bias", [P, 1], f32, isOutput=False)
    lin_w_p = nc.declare_dram_parameter("lin_w", [FILT, 1], f32, isOutput=False)
    lin_b_p = nc.declare_dram_parameter("lin_b", [1, 1], f32, isOutput=False)
    out_p = nc.declare_dram_parameter("out", [1, NL], f32, isOutput=True)

    bounceO = nc.dram_tensor("bounceO", [NL, CH], f32)
    bounceI = nc.dram_tensor("bounceI", [NL, CH], f32)
    tableO = nc.dram_tensor("tableO", [NT, CH], f32, addr_space="Shared")
    tableI = nc.dram_tensor("tableI", [NT, CH], f32, addr_space="Shared")
    txis_d = nc.dram_tensor("txis", [NL, CH], f32)

    replica_groups = [list(range(n_cores))]

    with ExitStack() as ctx:
        tc = ctx.enter_context(tile.TileContext(nc))
        persist = ctx.enter_context(tc.tile_pool(name="persist", bufs=1))
        work = ctx.enter_context(tc.tile_pool(name="work", bufs=2))
        gpool = ctx.enter_context(tc.tile_pool(name="gpool", bufs=3))
        psum = ctx.enter_context(tc.tile_pool(name="psum", bufs=2, space="PSUM"))
        psum_pre = ctx.enter_context(tc.tile_pool(name="psum_pre", bufs=2, space="PSUM"))

        # ---- persistent tiles & input DMAs ----
        xg = persist.tile([P, T, CH], f32)
        AT = persist.tile([FILT, NL], f32)
        idxA_t = persist.tile([P, WA], i32)
        idxB_t = persist.tile([P, WB], i32)
        wgA_t = persist.tile([P, WA], f32)
        wgB_t = persist.tile([P, cfg["WBd"]], f32)
        perm_t = persist.tile([P, T], i32)
        Wcat = persist.tile([FILT, P], f32)
        w_id0_t = persist.tile([CH, P], f32)
        w_id1_t = persist.tile([CH, P], f32)
        bias_t = persist.tile([P, 1], f32)
        bias_h = persist.tile([P, 1], f32)
        lin_w_t = persist.tile([FILT, 1], f32)
        lin_b_t = persist.tile([1, 1], f32)
        ident = persist.tile([P, P], f32)

        nc.vector.memset(AT[0:2 * CH, :], 0.0)
        nc.vector.memset(Wcat[0:2 * CH, :], 0.0)
        nc.sync.dma_start(out=wgA_t[:], in_=wgA_p[:])
        nc.sync.dma_start(out=wgB_t[:], in_=wgB_p[:])
        nc.sync.dma_start(out=xg[:], in_=x_grid_p[:])
        nc.sync.dma_start(out=AT[0:CH, :], in_=xT_p[:])
        nc.sync.dma_start(out=idxA_t[:], in_=idxA_p[:])
        nc.sync.dma_start(out=idxB_t[:], in_=idxB_p[:])
        nc.sync.dma_start(out=perm_t[:], in_=perm_p[:])
        nc.sync.dma_start(out=w_id0_t[:], in_=w_id0_p[:])
        nc.sync.dma_start(out=w_id1_t[:], in_=w_id1_p[:])
        nc.sync.dma_start(out=Wcat[2 * CH:4 * CH, :], in_=w_dif_p[:])
        nc.sync.dma_start(out=bias_t[:], in_=bias_p[:])
        nc.sync.dma_start(out=lin_w_t[:], in_=lin_w_p[:])
        nc.sync.dma_start(out=lin_b_t[:], in_=lin_b_p[:])
        make_identity(nc, ident[:])

        nc.vector.tensor_add(out=Wcat[0:CH, :], in0=w_id0_t[:], in1=w_id1_t[:])
        # bias halves: Z-part scaled by 0.5 for the tanh-based sigmoid
        nc.vector.tensor_scalar_mul(out=bias_h[0:FILT, :], in0=bias_t[0:FILT, :],
                                    scalar1=0.5)
        nc.vector.tensor_copy(out=bias_h[FILT:P, :], in_=bias_t[FILT:P, :])

        # ---- phase 1: degrees (both in pi order; no permute needed) ----
        degI = persist.tile([P, T], f32)
        degO = persist.tile([P, T, 1], f32)
        for wg_t, deg, chunks in ((wgA_t, degI[:, :], cfg["chunksA"]),
                                  (wgB_t, degO[:, :, 0], cfg["chunksBd"])):
            for (t0, t1, o0, o1, runs) in chunks:
                ro = o0
                for (r0, r1, D) in runs:
                    nt = r1 - r0
                    nc.vector.tensor_reduce(
                        out=deg[:, r0:r1],
                        in_=wg_t[:, ro:ro + nt * D].rearrange(
                            "p (t d) -> p t d", t=nt),
                        axis=mybir.AxisListType.X, op=mybir.AluOpType.add)
                    ro += nt * D

        rin = persist.tile([P, T], f32)
        rout = persist.tile([P, T], f32)
        nc.vector.tensor_scalar_max(out=rin[:], in0=degI[:], scalar1=1e-30)
        nc.vector.reciprocal(out=rin[:], in_=rin[:])
        nc.vector.tensor_scalar_max(out=rout[:], in0=degO[:, :, 0], scalar1=1e-30)
        nc.vector.reciprocal(out=rout[:], in_=rout[:])

        # ---- phase 2: xs tables + two AllGathers ----
        # xs_i first: the B-direction storm runs first and only needs tableI;
        # the tableO AllGather then hides under the running B storm.
        xsI = persist.tile([P, T, CH], f32)
        xsO = persist.tile([P, T, CH], f32)
        nc.vector.tensor_tensor(out=xsI[:], in0=xg[:],
                                in1=rin[:].to_broadcast([P, T, CH]),
                                op=mybir.AluOpType.mult)
        nc.sync.dma_start(out=bounceI[:], in_=xsI[:])
        nc.gpsimd.collective_compute(
            "AllGather", mybir.AluOpType.bypass,
            replica_groups=replica_groups,
            ins=[bounceI[:]], outs=[tableI[:]])
        nc.vector.tensor_tensor(out=xsO[:], in0=xg[:],
                                in1=rout[:].to_broadcast([P, T, CH]),
                                op=mybir.AluOpType.mult)
        nc.sync.dma_start(out=bounceO[:], in_=xsO[:])
        nc.gpsimd.collective_compute(
            "AllGather", mybir.AluOpType.bypass,
            replica_groups=replica_groups,
            ins=[bounceO[:]], outs=[tableO[:]])

        # ---- phase 3: gathers + segmented reduces ----
        # HW indirect DMA semantics: one descriptor per partition, reading
        # out.free_size contiguous elements from idx[p, 0]*coef. So each
        # instruction gathers one grid column: G[:, w, :] = table[idxX[:, w]].
        # TxC channels 0:16 = TxO (pi order), 16:32 = TxI (permuted in below)
        TxC = persist.tile([P, T, 2 * CH], f32)
        TxIs = persist.tile([P, T, CH], f32)
        def storm(idx_t, Tx, coff, chunks, table):
            for (t0, t1, o0, o1, runs) in chunks:
                wchunk = o1 - o0
                G = gpool.tile([P, 192, CH], f32, tag="gbuf")
                for w in range(wchunk):
                    nc.gpsimd.indirect_dma_start(
                        out=G[:, w, :], out_offset=None, in_=table[:],
                        in_offset=bass.IndirectOffsetOnAxis(
                            ap=idx_t[:, o0 + w:o0 + w + 1], axis=0))
                ro = 0
                for (r0, r1, D) in runs:
                    nt = r1 - r0
                    out_ap = (Tx[:, r0:r1, 0:CH] if coff is not None
                              else Tx[:, r0:r1, :])
                    nc.vector.tensor_reduce(
                        out=out_ap,
                        in_=G[:, ro:ro + nt * D, :].rearrange(
                            "p (t d) c -> p t c d", t=nt),
                        axis=mybir.AxisListType.X, op=mybir.AluOpType.add)
                    ro += nt * D
        storm(idxB_t, TxIs, None, cfg["chunksB"], tableI)
        storm(idxA_t, TxC, 0, cfg["chunksA"], tableO)
        # permute TxI sigma->pi through DRAM, landing in TxC channels 16:32
        nc.sync.dma_start(out=txis_d[:], in_=TxIs[:])
        for t in range(T):
            nc.gpsimd.indirect_dma_start(
                out=TxC[:, t, CH:2 * CH], out_offset=None, in_=txis_d[:],
                in_offset=bass.IndirectOffsetOnAxis(
                    ap=perm_t[:, t:t + 1], axis=0))

        # ---- phase 4: transposes into AT rows 32:64 ----
        # 4 tiles per transpose: out rows 32*i:32*i+32 = tile (g0+i) [TxO|TxI]
        for g0 in range(0, T, 4):
            nt = min(4, T - g0)
            ps = psum.tile([P, P], f32, tag="tps")
            nc.tensor.transpose(
                out=ps[0:nt * 2 * CH, :],
                in_=TxC[:, g0:g0 + nt, :].rearrange("p t c -> p (t c)"),
                identity=ident[:])
            for i in range(nt):
                nc.scalar.copy(
                    out=AT[2 * CH:4 * CH, (g0 + i) * P:(g0 + i + 1) * P],
                    in_=ps[i * 2 * CH:(i + 1) * 2 * CH, :])

        # ---- phase 5: epilogue ----
        out_sb = persist.tile([1, NL], f32)
        CW = 512
        nchunks = (NL + CW - 1) // CW
        for c in range(nchunks):
            lo = c * CW
            w = min(CW, NL - lo)
            pre = psum_pre.tile([P, CW], f32, tag="pre")
            nc.tensor.matmul(out=pre[:, 0:w], lhsT=Wcat[:], rhs=AT[:, lo:lo + w],
                             start=True, stop=True)
            z = work.tile([FILT, CW], f32, tag="z")
            ht = work.tile([FILT, CW], f32, tag="ht")
            # sigmoid(x) = 0.5*tanh(0.5*x) + 0.5  (single ACT table)
            nc.scalar.activation(out=z[:, 0:w], in_=pre[0:FILT, 0:w],
                                 func=mybir.ActivationFunctionType.Tanh,
                                 bias=bias_h[0:FILT, :], scale=0.5)
            nc.scalar.activation(out=ht[:, 0:w], in_=pre[FILT:P, 0:w],
                                 func=mybir.ActivationFunctionType.Tanh,
                                 bias=bias_h[FILT:P, :], scale=1.0)
            nc.vector.tensor_scalar(out=z[:, 0:w], in0=z[:, 0:w],
                                    scalar1=0.5, scalar2=0.5,
                                    op0=mybir.AluOpType.mult,
                                    op1=mybir.AluOpType.add)
            h = work.tile([FILT, CW], f32, tag="h")
            nc.vector.tensor_mul(out=h[:, 0:w], in0=z[:, 0:w], in1=ht[:, 0:w])
            nc.vector.tensor_tensor(out=h[:, 0:w], in0=ht[:, 0:w], in1=h[:, 0:w],
                                    op=mybir.AluOpType.subtract)
            nc.vector.tensor_scalar_max(out=h[:, 0:w], in0=h[:, 0:w], scalar1=0.0)
            ps2 = psum.tile([1, CW], f32, tag="ps2")
            nc.tensor.matmul(out=ps2[:, 0:w], lhsT=lin_w_t[:], rhs=h[:, 0:w],
                             start=True, stop=True)
            nc.vector.tensor_scalar_add(out=out_sb[:, lo:lo + w], in0=ps2[:, 0:w],
                                        scalar1=lin_b_t[0:1, :])
        nc.sync.dma_start(out=out_p[:], in_=out_sb[:])

        if debug:
            nc.sync.dma_start(out=dbg["d_degI"][:], in_=degI[:])
            nc.sync.dma_start(out=dbg["d_degO"][:], in_=degO[:, :, 0])
            nc.sync.dma_start(out=dbg["d_TxC"][:], in_=TxC[:])
            nc.sync.dma_start(out=dbg["d_AT"][:], in_=AT[:])
            nc.sync.dma_start(out=dbg["d_Wcat"][:], in_=Wcat[:])

    nc.compile()
    return nc


# ---------------------------------------------------------------------------
# harness entry point
# ---------------------------------------------------------------------------

_CACHE = {}


def kernel(x, edge_index, edge_weight, w_z, b_z, w_r, b_r, w_h, b_h, lin_w, lin_b):
    """Distributed DCRNN forward on 8 TRN2 NeuronCores.

    Takes full unsharded inputs, returns the full [N, 1] float32 output.
    (w_r/b_r are dead inputs: H0 = 0 makes the reset gate a no-op.)
    """
    from concourse.bass_utils import run_bass_kernel_spmd

    x = np.ascontiguousarray(np.asarray(x, dtype=np.float32))
    cfg, arrays = preprocess(x, np.asarray(edge_index), np.asarray(edge_weight),
                             n_cores=8)
    in_maps = make_in_maps(cfg, arrays, np.asarray(w_z, np.float32),
                           np.asarray(b_z, np.float32),
                           np.asarray(w_h, np.float32),
                           np.asarray(b_h, np.float32),
                           np.asarray(lin_w, np.float32),
                           np.asarray(lin_b, np.float32))
    key = (cfg["N"], cfg["E"], cfg["WA"], cfg["WB"], cfg["WBd"],
           tuple(tuple(c[:4]) for c in cfg["chunksA"]),
           tuple(tuple(c[:4]) for c in cfg["chunksB"]))
    nc = _CACHE.get(key)
    if nc is None:
        nc = build_kernel(cfg)
        _CACHE[key] = nc
    res = run_bass_kernel_spmd(nc, in_maps, core_ids=list(range(8)))
    return postprocess(cfg, arrays, res.results)



# revision 3
# speedup vs baseline: 8.6495x; 8.6495x over previous
"""Trainium2 Bass kernel for the distributed DCRNN (gnn_message_passing) problem.

Strategy: node-shard across 8 cores (dealt by in-degree rank so all cores share
one compiled grid geometry). All graph indirection is resolved HOST-side by
duplicating INPUT data per edge cell (pure index plumbing — no host arithmetic):

  - xdup[cell]  = x[src(cell)]          (bf16)
  - wdup[cell]  = src's full weight list (bf16, padded to K)

The device then re-derives the per-edge scale on-chip (deg = reduce(wdup),
s = 1/deg), forms messages m = xdup * s, and segment-reduces per destination.
Everything moves via regular strided HWDGE DMAs at full bandwidth — no
indirect/SWDGE descriptors (measured at ~8.3ns/descriptor, they were the
baseline bottleneck), no collectives, no halo exchange.
"""

from contextlib import ExitStack

import ml_dtypes
import numpy as np

import concourse.bass as bass
import concourse.bacc as bacc
import concourse.mybir as mybir
import concourse.tile as tile
from concourse.masks import make_identity

P = 128
CH = 16
FILT = 64
CWMAX = 192    # grid columns per streamed chunk


# ---------------------------------------------------------------------------
# host-side preprocessing (index plumbing only; no reference arithmetic)
# ---------------------------------------------------------------------------

def chunk_plan(D, max_width=CWMAX):
    """Split tiles into chunks (contiguous tiles, bounded total width) and
    equal-width runs within each chunk: (t_lo, t_hi, off_lo, off_hi, runs)."""
    T = len(D)
    off = np.concatenate([[0], np.cumsum(D)]).astype(np.int64)
    chunks = []
    t = 0
    while t < T:
        t0 = t
        w = 0
        while t < T and (w + D[t] <= max_width or t == t0):
            w += D[t]
            t += 1
        runs = []
        r = t0
        while r < t:
            r0 = r
            while r < t and D[r] == D[r0]:
                r += 1
            runs.append((r0, r, int(D[r0])))
        chunks.append((t0, t, int(off[t0]), int(off[t]), runs))
    return chunks


def preprocess(x, edge_index, edge_weight, n_cores=8):
    N = x.shape[0]
    E = edge_index.shape[1]
    NPC = N // n_cores
    T = (NPC + P - 1) // P
    NL = P * T
    row = np.ascontiguousarray(edge_index[0]).astype(np.int64)
    col = np.ascontiguousarray(edge_index[1]).astype(np.int64)
    w = np.ascontiguousarray(edge_weight).astype(np.float32)

    cnt_in = np.bincount(col, minlength=N)
    cnt_out = np.bincount(row, minlength=N)

    # deal nodes to cores by global in-degree rank so per-tile degree profiles
    # match across cores (one compiled kernel; minimal tile-width padding)
    g_order = np.argsort(cnt_in, kind="stable")
    cores = np.empty(N, dtype=np.int64)
    cores[g_order] = np.arange(N) % n_cores

    perm = np.full((n_cores, NL), -1, dtype=np.int64)
    pos = np.empty(N, dtype=np.int64)
    for k in range(n_cores):
        nodes = np.where(cores == k)[0]
        order = np.argsort(cnt_in[nodes], kind="stable")
        perm[k, :NPC] = nodes[order]
        pos[nodes[order]] = np.arange(NPC)

    def tile_widths(cnt):
        D = np.zeros(T, dtype=np.int64)
        for k in range(n_cores):
            c = np.where(perm[k] >= 0, cnt[np.maximum(perm[k], 0)], 0)
            D = np.maximum(D, c.reshape(T, P).max(axis=1))
        return np.maximum(D, 1)

    DA = tile_widths(cnt_in)    # A-grid: in-edges per dest (dest = col)
    DB = tile_widths(cnt_out)   # B-grid: out-edges per dest (dest = row)
    offA = np.concatenate([[0], np.cumsum(DA)]).astype(np.int64)
    offB = np.concatenate([[0], np.cumsum(DB)]).astype(np.int64)
    WA, WB = int(offA[-1]), int(offB[-1])

    KA = int(cnt_out.max())     # A scale = 1/deg_out(src): src weight list len
    KB = int(cnt_in.max())      # B scale = 1/deg_in(src)

    # padded per-node weight lists (bf16)
    def weight_lists(key, K):
        wp = np.zeros((N, K), dtype=ml_dtypes.bfloat16)
        order = np.argsort(key, kind="stable")
        ks = key[order]
        start = np.concatenate([[0], np.cumsum(np.bincount(ks, minlength=N))])[ks]
        slot = np.arange(E) - start
        wp[ks, slot] = w[order].astype(ml_dtypes.bfloat16)
        return wp
    w_out_pad = weight_lists(row, KA)
    w_in_pad = weight_lists(col, KB)

    xbf = np.asarray(x, dtype=np.float32).astype(ml_dtypes.bfloat16)

    def build_dup(dest, src, off, W, w_pad, K):
        xdup = np.zeros((n_cores, P, W, CH), dtype=ml_dtypes.bfloat16)
        wdup = np.zeros((n_cores, P, W, K), dtype=ml_dtypes.bfloat16)
        k_e = cores[dest]
        j_e = pos[dest]
        t_e, p_e = j_e // P, j_e % P
        order = np.argsort(dest, kind="stable")
        ds = dest[order]
        start = np.concatenate([[0], np.cumsum(np.bincount(ds, minlength=N))])[ds]
        s_e = np.empty(E, dtype=np.int64)
        s_e[order] = np.arange(E) - start
        wcol = off[t_e] + s_e
        xdup[k_e, p_e, wcol] = xbf[src]
        wdup[k_e, p_e, wcol] = w_pad[src]
        return xdup, wdup

    xdupA, wdupA = build_dup(col, row, offA, WA, w_out_pad, KA)
    xdupB, wdupB = build_dup(row, col, offB, WB, w_in_pad, KB)

    xT = np.zeros((n_cores, CH, NL), dtype=np.float32)
    for k in range(n_cores):
        pk = perm[k]
        valid = pk >= 0
        xg = np.zeros((NL, CH), dtype=np.float32)
        xg[valid] = np.asarray(x, dtype=np.float32)[pk[valid]]
        xT[k] = xg.T

    cfg = dict(
        N=N, E=E, NPC=NPC, T=T, NL=NL, WA=WA, WB=WB, KA=KA, KB=KB,
        n_cores=n_cores, chunksA=chunk_plan(DA), chunksB=chunk_plan(DB),
    )
    arrays = dict(
        xdupA=xdupA, wdupA=wdupA, xdupB=xdupB, wdupB=wdupB, xT=xT, permA=perm,
    )
    return cfg, arrays


def make_in_maps(cfg, arrays, w_z, b_z, w_h, b_h, lin_w, lin_b):
    """AT row layout: [x^T (0:16) | zeros (16:32) | TxO^T (32:48) | TxI^T (48:64)].
    Wcat rows match; rows 16:32 are zero (contraction-dim padding is free)."""
    n_cores = cfg["n_cores"]
    w_id0 = np.concatenate([w_z[0, 0, :CH], w_h[0, 0, :CH]], axis=1).astype(np.float32)
    w_id1 = np.concatenate([w_z[1, 0, :CH], w_h[1, 0, :CH]], axis=1).astype(np.float32)
    w_dif = np.concatenate(
        [np.concatenate([w_z[0, 1, :CH], w_h[0, 1, :CH]], axis=1),
         np.concatenate([w_z[1, 1, :CH], w_h[1, 1, :CH]], axis=1)],
        axis=0).astype(np.float32)
    bias = np.concatenate([b_z, b_h]).astype(np.float32).reshape(P, 1)
    in_maps = []
    for k in range(n_cores):
        in_maps.append({
            "xT": np.ascontiguousarray(arrays["xT"][k]),
            "xdupA": np.ascontiguousarray(arrays["xdupA"][k]),
            "wdupA": np.ascontiguousarray(arrays["wdupA"][k]),
            "xdupB": np.ascontiguousarray(arrays["xdupB"][k]),
            "wdupB": np.ascontiguousarray(arrays["wdupB"][k]),
            "w_id0": w_id0, "w_id1": w_id1, "w_dif": w_dif,
            "bias": bias,
            "lin_w": lin_w.astype(np.float32),
            "lin_b": lin_b.astype(np.float32).reshape(1, 1),
        })
    return in_maps


def postprocess(cfg, arrays, results):
    """results[k]['out'] is [1, NL]; scatter back to [N, 1] full output."""
    N, NL = cfg["N"], cfg["NL"]
    out = np.zeros((N, 1), dtype=np.float32)
    for k in range(cfg["n_cores"]):
        o = np.asarray(results[k]["out"]).reshape(NL)
        pk = arrays["permA"][k]
        valid = pk >= 0
        out[pk[valid], 0] = o[valid]
    return out


# ---------------------------------------------------------------------------
# device kernel
# ---------------------------------------------------------------------------

def build_kernel(cfg, debug=False):
    T, NL, WA, WB = cfg["T"], cfg["NL"], cfg["WA"], cfg["WB"]
    KA, KB = cfg["KA"], cfg["KB"]
    f32 = mybir.dt.float32
    bf16 = mybir.dt.bfloat16

    nc = bacc.Bacc()

    xT_p = nc.declare_dram_parameter("xT", [CH, NL], f32, isOutput=False)
    xdupA_p = nc.declare_dram_parameter("xdupA", [P, WA, CH], bf16, isOutput=False)
    wdupA_p = nc.declare_dram_parameter("wdupA", [P, WA, KA], bf16, isOutput=False)
    xdupB_p = nc.declare_dram_parameter("xdupB", [P, WB, CH], bf16, isOutput=False)
    wdupB_p = nc.declare_dram_parameter("wdupB", [P, WB, KB], bf16, isOutput=False)
    w_id0_p = nc.declare_dram_parameter("w_id0", [CH, P], f32, isOutput=False)
    w_id1_p = nc.declare_dram_parameter("w_id1", [CH, P], f32, isOutput=False)
    w_dif_p = nc.declare_dram_parameter("w_dif", [2 * CH, P], f32, isOutput=False)
    bias_p = nc.declare_dram_parameter("bias", [P, 1], f32, isOutput=False)
    lin_w_p = nc.declare_dram_parameter("lin_w", [FILT, 1], f32, isOutput=False)
    lin_b_p = nc.declare_dram_parameter("lin_b", [1, 1], f32, isOutput=False)
    out_p = nc.declare_dram_parameter("out", [1, NL], f32, isOutput=True)

    with ExitStack() as ctx:
        tc = ctx.enter_context(tile.TileContext(nc))
        persist = ctx.enter_context(tc.tile_pool(name="persist", bufs=1))
        wpool = ctx.enter_context(tc.tile_pool(name="wpool", bufs=2))
        xpool = ctx.enter_context(tc.tile_pool(name="xpool", bufs=2))
        spool = ctx.enter_context(tc.tile_pool(name="spool", bufs=2))
        mpool = ctx.enter_context(tc.tile_pool(name="mpool", bufs=2))
        work = ctx.enter_context(tc.tile_pool(name="work", bufs=2))
        psum = ctx.enter_context(tc.tile_pool(name="psum", bufs=2, space="PSUM"))
        psum_pre = ctx.enter_context(tc.tile_pool(name="psum_pre", bufs=2, space="PSUM"))

        # ---- persistent tiles & input DMAs ----
        AT = persist.tile([FILT, NL], f32)
        TxC = persist.tile([P, T, 2 * CH], f32)
        Wcat = persist.tile([FILT, P], f32)
        w_id0_t = persist.tile([CH, P], f32)
        w_id1_t = persist.tile([CH, P], f32)
        bias_t = persist.tile([P, 1], f32)
        bias_h = persist.tile([P, 1], f32)
        lin_w_t = persist.tile([FILT, 1], f32)
        lin_b_t = persist.tile([1, 1], f32)
        ident = persist.tile([P, P], f32)

        nc.vector.memset(AT[0:2 * CH, :], 0.0)
        nc.vector.memset(Wcat[0:2 * CH, :], 0.0)
        nc.sync.dma_start(out=AT[0:CH, :], in_=xT_p[:])
        nc.sync.dma_start(out=w_id0_t[:], in_=w_id0_p[:])
        nc.sync.dma_start(out=w_id1_t[:], in_=w_id1_p[:])
        nc.sync.dma_start(out=Wcat[2 * CH:4 * CH, :], in_=w_dif_p[:])
        nc.sync.dma_start(out=bias_t[:], in_=bias_p[:])
        nc.sync.dma_start(out=lin_w_t[:], in_=lin_w_p[:])
        nc.sync.dma_start(out=lin_b_t[:], in_=lin_b_p[:])
        make_identity(nc, ident[:])

        nc.vector.tensor_add(out=Wcat[0:CH, :], in0=w_id0_t[:], in1=w_id1_t[:])
        # bias halves: Z-part scaled by 0.5 for the tanh-based sigmoid
        nc.vector.tensor_scalar_mul(out=bias_h[0:FILT, :], in0=bias_t[0:FILT, :],
                                    scalar1=0.5)
        nc.vector.tensor_copy(out=bias_h[FILT:P, :], in_=bias_t[FILT:P, :])

        # ---- streamed message passing (both directions) ----
        # per chunk: load [P, Wc, K] weights + [P, Wc, CH] features; on-chip
        # deg = reduce(w), s = 1/deg, m = x * s, segment-reduce into TxC.
        for (xdup_p, wdup_p, Kd, chunks, ch0) in (
                (xdupA_p, wdupA_p, KA, cfg["chunksA"], 0),
                (xdupB_p, wdupB_p, KB, cfg["chunksB"], CH)):
            for (t0, t1, o0, o1, runs) in chunks:
                Wc = o1 - o0
                wd = wpool.tile([P, CWMAX, Kd], bf16, tag="wd")
                nc.sync.dma_start(out=wd[:, 0:Wc, :], in_=wdup_p[:, o0:o1, :])
                xd = xpool.tile([P, CWMAX, CH], bf16, tag="xd")
                nc.scalar.dma_start(out=xd[:, 0:Wc, :], in_=xdup_p[:, o0:o1, :])
                s = spool.tile([P, CWMAX], f32, tag="s")
                nc.vector.tensor_reduce(out=s[:, 0:Wc], in_=wd[:, 0:Wc, :],
                                        axis=mybir.AxisListType.X,
                                        op=mybir.AluOpType.add)
                nc.vector.tensor_scalar_max(out=s[:, 0:Wc], in0=s[:, 0:Wc],
                                            scalar1=1e-30)
                nc.vector.reciprocal(out=s[:, 0:Wc], in_=s[:, 0:Wc])
                m = mpool.tile([P, CWMAX, CH], f32, tag="m")
                nc.vector.tensor_tensor(out=m[:, 0:Wc, :], in0=xd[:, 0:Wc, :],
                                        in1=s[:, 0:Wc].to_broadcast([P, Wc, CH]),
                                        op=mybir.AluOpType.mult)
                ro = 0
                for (r0, r1, D) in runs:
                    nt = r1 - r0
                    nc.vector.tensor_reduce(
                        out=TxC[:, r0:r1, ch0:ch0 + CH],
                        in_=m[:, ro:ro + nt * D, :].rearrange(
                            "p (t d) c -> p t c d", t=nt),
                        axis=mybir.AxisListType.X, op=mybir.AluOpType.add)
                    ro += nt * D

        # ---- transposes into AT rows 32:64 ----
        # 4 tiles per transpose: out rows 32*i:32*i+32 = tile (g0+i) [TxO|TxI]
        for g0 in range(0, T, 4):
            nt = min(4, T - g0)
            ps = psum.tile([P, P], f32, tag="tps")
            nc.tensor.transpose(
                out=ps[0:nt * 2 * CH, :],
                in_=TxC[:, g0:g0 + nt, :].rearrange("p t c -> p (t c)"),
                identity=ident[:])
            for i in range(nt):
                nc.scalar.copy(
                    out=AT[2 * CH:4 * CH, (g0 + i) * P:(g0 + i + 1) * P],
                    in_=ps[i * 2 * CH:(i + 1) * 2 * CH, :])

        # ---- epilogue ----
        out_sb = persist.tile([1, NL], f32)
        CW = 512
        nchunks = (NL + CW - 1) // CW
        for c in range(nchunks):
            lo = c * CW
            w = min(CW, NL - lo)
            pre = psum_pre.tile([P, CW], f32, tag="pre")
            nc.tensor.matmul(out=pre[:, 0:w], lhsT=Wcat[:], rhs=AT[:, lo:lo + w],
                             start=True, stop=True)
            z = work.tile([FILT, CW], f32, tag="z")
            ht = work.tile([FILT, CW], f32, tag="ht")
            # sigmoid(x) = 0.5*tanh(0.5*x) + 0.5  (single ACT table)
            nc.scalar.activation(out=z[:, 0:w], in_=pre[0:FILT, 0:w],
                                 func=mybir.ActivationFunctionType.Tanh,
                                 bias=bias_h[0:FILT, :], scale=0.5)
            nc.scalar.activation(out=ht[:, 0:w], in_=pre[FILT:P, 0:w],
                                 func=mybir.ActivationFunctionType.Tanh,
                                 bias=bias_h[FILT:P, :], scale=1.0)
            nc.vector.tensor_scalar(out=z[:, 0:w], in0=z[:, 0:w],
                                    scalar1=0.5, scalar2=0.5,
                                    op0=mybir.AluOpType.mult,
                                    op1=mybir.AluOpType.add)
            h = work.tile([FILT, CW], f32, tag="h")
            nc.vector.tensor_mul(out=h[:, 0:w], in0=z[:, 0:w], in1=ht[:, 0:w])
            nc.vector.tensor_tensor(out=h[:, 0:w], in0=ht[:, 0:w], in1=h[:, 0:w],
                                    op=mybir.AluOpType.subtract)
            nc.vector.tensor_scalar_max(out=h[:, 0:w], in0=h[:, 0:w], scalar1=0.0)
            ps2 = psum.tile([1, CW], f32, tag="ps2")
            nc.tensor.matmul(out=ps2[:, 0:w], lhsT=lin_w_t[:], rhs=h[:, 0:w],
                             start=True, stop=True)
            nc.vector.tensor_scalar_add(out=out_sb[:, lo:lo + w], in0=ps2[:, 0:w],
                                        scalar1=lin_b_t[0:1, :])
        nc.sync.dma_start(out=out_p[:], in_=out_sb[:])

    nc.compile()
    return nc


# ---------------------------------------------------------------------------
# harness entry point
# ---------------------------------------------------------------------------

_CACHE = {}


def kernel(x, edge_index, edge_weight, w_z, b_z, w_r, b_r, w_h, b_h, lin_w, lin_b):
    """Distributed DCRNN forward on 8 TRN2 NeuronCores.

    Takes full unsharded inputs, returns the full [N, 1] float32 output.
    (w_r/b_r are dead inputs: H0 = 0 makes the reset gate a no-op.)
    """
    from concourse.bass_utils import run_bass_kernel_spmd

    x = np.ascontiguousarray(np.asarray(x, dtype=np.float32))
    cfg, arrays = preprocess(x, np.asarray(edge_index), np.asarray(edge_weight),
                             n_cores=8)
    in_maps = make_in_maps(cfg, arrays, np.asarray(w_z, np.float32),
                           np.asarray(b_z, np.float32),
                           np.asarray(w_h, np.float32),
                           np.asarray(b_h, np.float32),
                           np.asarray(lin_w, np.float32),
                           np.asarray(lin_b, np.float32))
    key = (cfg["N"], cfg["E"], cfg["WA"], cfg["WB"], cfg["KA"], cfg["KB"],
           tuple(tuple(c[:4]) for c in cfg["chunksA"]),
           tuple(tuple(c[:4]) for c in cfg["chunksB"]))
    nc = _CACHE.get(key)
    if nc is None:
        nc = build_kernel(cfg)
        _CACHE[key] = nc
    res = run_bass_kernel_spmd(nc, in_maps, core_ids=list(range(8)))
    return postprocess(cfg, arrays, res.results)
